# revision 4
# baseline (speedup 1.0000x reference)
"""ResNet BasicBlock (conv3x3-BN-ReLU-conv3x3-BN-add-ReLU) on 8 Trainium2 cores.

Strategy:
  - Pure data parallel: batch 32 -> 4 images per core; weights/BN replicated.
  - BN folded into conv weights on host; bias applied in the epilogue.
  - 1D Winograd F(2,3) along W: 1.5x fewer PE MACs than direct conv.
      V_b = (B^T d)_b  computed on DVE as slice add/subs of the padded input
      m_b = sum_{ky,ib} wtil_b[ky]^T @ V_b   (PE, PSUM fp32; 24 matmuls x
            392 cols per 14-row chunk vs direct 18 x 784)
      y_even = m0+m1+m2, y_odd = m1-m2-m3   (Scalar copies the shared m1/m2
            to SBUF fp16; DVE/Pool combine; m0/m3 read straight from PSUM)
    Weight transform (G along kx) and BN fold are done on host.
  - Columns stored DE-INTERLEAVED (all 29 even cols, then all 29 odd cols)
    for x and h, so every V-transform/residual slice is contiguous fp16 ->
    DVE runs them in 2x_1p mode (the only fast mode TensorTensor supports).
  - Epilogue combine ops that touch PSUM (fp32) stay on the DVE; the
    fp16-only ones (o1, e, residual adds) go to the otherwise-idle Pool
    (gpsimd) engine so the DVE keeps up with the PE.
  - fp16 matmul operands, fp32 PSUM accumulation (rel err ~1e-3, gate 2e-2).
"""

import numpy as np

import concourse.mybir as mybir
import concourse.tile as tile
from concourse import bacc
from concourse.bass_utils import run_bass_kernel_spmd

EPS = 1e-5
NCORES = 8
N, C, H, W = 32, 256, 56, 56
NPC = N // NCORES          # images per core
HP, WP = H + 2, W + 2      # padded spatial
CB = C // 128              # channel blocks (2)
RC = 14                    # rows per PSUM chunk
NCHUNK = H // RC           # 4 chunks
T = W // 2                 # winograd tiles per row (28)
F16 = mybir.dt.float16
F32 = mybir.dt.float32

_CACHE = {}


def _build():
    nc = bacc.Bacc("TRN2", target_bir_lowering=False, debug=False,
                   num_devices=NCORES)
    xp = nc.dram_tensor("xp", [NPC, CB, 128, HP, WP], F16,
                        kind="ExternalInput").ap()
    w1t = nc.dram_tensor("w1t", [CB, 128, 12, C], F16, kind="ExternalInput").ap()
    w2t = nc.dram_tensor("w2t", [CB, 128, 12, C], F16, kind="ExternalInput").ap()
    b1 = nc.dram_tensor("b1", [CB, 128, 1], F32, kind="ExternalInput").ap()
    b2 = nc.dram_tensor("b2", [CB, 128, 1], F32, kind="ExternalInput").ap()
    y = nc.dram_tensor("y", [NPC, CB, 128, H, W], F32,
                       kind="ExternalOutput").ap()

    Relu = mybir.ActivationFunctionType.Relu
    Copy = mybir.ActivationFunctionType.Copy
    Add = mybir.AluOpType.add
    Sub = mybir.AluOpType.subtract

    # de-interleaved column regions of a padded 58-col row:
    # cols [0:29) = original even cols 0,2,..,56 ("E"), [29:58) = odd 1,..,57
    EO0, EO1 = slice(0, 28), slice(1, 29)        # E[t], E[t+1]
    OO0, OO1 = slice(29, 57), slice(30, 58)      # O[t], O[t+1]

    with tile.TileContext(nc) as tc:
        with tc.tile_pool(name="w", bufs=1) as wp, \
             tc.tile_pool(name="x", bufs=3) as xpool, \
             tc.tile_pool(name="v", bufs=1) as vpool, \
             tc.tile_pool(name="h", bufs=1) as hpool, \
             tc.tile_pool(name="yst", bufs=3) as ypool, \
             tc.tile_pool(name="tmp", bufs=3) as tpool, \
             tc.tile_pool(name="ps", bufs=8, space="PSUM") as pspool:

            # Startup: DMA issues serialize at ~620ns each on the Sync queue,
            # so order by first-need: x halves ib-interleaved (they feed the
            # V transform, the longest dependency chain), then weights.
            hh = HP // 2
            w1s, w2s, b1s, b2s = [], [], [], []
            xt0 = [xpool.tile([128, HP, WP], F16, tag=f"x{ib}", name=f"xt0_{ib}")
                   for ib in range(CB)]
            for ib in range(CB):
                nc.sync.dma_start(out=xt0[ib][:, :hh, :],
                                  in_=xp[0, ib, :, :hh, :])
            for ib in range(CB):
                nc.sync.dma_start(out=xt0[ib][:, hh:, :],
                                  in_=xp[0, ib, :, hh:, :])
            for ib in range(CB):
                t = wp.tile([128, 12, C], F16, tag=f"w1_{ib}")
                w1s.append(t)
                nc.sync.dma_start(out=w1s[ib][:, :, :128],
                                  in_=w1t[ib, :, :, :128])
            for ib in range(CB):
                nc.sync.dma_start(out=w1s[ib][:, :, 128:],
                                  in_=w1t[ib, :, :, 128:])
                t = wp.tile([128, 1], F32, tag=f"b1_{ib}")
                nc.sync.dma_start(out=t[:], in_=b1[ib])
                b1s.append(t)

            def load_w2():
                for ib in range(CB):
                    t = wp.tile([128, 12, C], F16, tag=f"w2_{ib}")
                    nc.sync.dma_start(out=t[:], in_=w2t[ib])
                    w2s.append(t)
                    t = wp.tile([128, 1], F32, tag=f"b2_{ib}")
                    nc.sync.dma_start(out=t[:], in_=b2[ib])
                    b2s.append(t)

            # PE warmup: the HAM clock gate holds the PE at 1.2 GHz until it
            # has been busy ~3.4us. The PE is idle during the initial DMA +
            # V-transform wait anyway, so run throwaway matmuls on a zeroed
            # scratch tile to unthrottle the clock before the first real one.
            scratch = wp.tile([128, RC * T], F16, tag="warm_scratch")
            nc.gpsimd.memset(scratch[:], 0.0)
            ps_w = pspool.tile([128, RC * T], F32, name="ps_warm", tag="ps")
            for _ in range(16):
                nc.tensor.matmul(ps_w[:], scratch[:, :128], scratch[:],
                                 start=True, stop=True)

            # persistent conv1-output tiles (de-interleaved cols, padded),
            # 2 channel blocks x 2 pipeline parities, fully zeroed ONCE (the
            # border cols/rows must stay zero); interior rewritten per image.
            hts_all = {}
            for par in range(2):
                for ob in range(CB):
                    t = hpool.tile([128, HP, WP], F16, tag=f"h{par}_{ob}")
                    nc.gpsimd.memset(t[:], 0.0)
                    hts_all[(par, ob)] = t

            # persistent Winograd-transform tiles (single-buffered: the
            # transform for the next use WAR-waits on the previous conv's
            # reads; the DVE has slack to hide that)
            v1s = [vpool.tile([128, 4, HP, T], F16, tag=f"v1_{ib}",
                              name=f"v1_{ib}") for ib in range(CB)]
            v2s = [vpool.tile([128, 4, HP, T], F16, tag=f"v2_{ib}",
                              name=f"v2_{ib}") for ib in range(CB)]

            def vtransform(vt, src, r0=0, r1=HP):
                # V_b = (B^T d)_b over W for padded rows [r0, r1), as
                # contiguous slice ops on the de-interleaved layout
                # (fp16 packed -> DVE 2x_1p mode)
                r = slice(r0, r1)
                for ib in range(CB):
                    s, v = src[ib], vt[ib]
                    nc.vector.tensor_tensor(out=v[:, 0, r], in0=s[:, r, EO0],
                                            in1=s[:, r, EO1], op=Sub)
                    nc.vector.tensor_tensor(out=v[:, 1, r], in0=s[:, r, OO0],
                                            in1=s[:, r, EO1], op=Add)
                    nc.vector.tensor_tensor(out=v[:, 2, r], in0=s[:, r, EO1],
                                            in1=s[:, r, OO0], op=Sub)
                    nc.vector.tensor_tensor(out=v[:, 3, r], in0=s[:, r, OO0],
                                            in1=s[:, r, OO1], op=Sub)

            def load_x(img):
                xt = []
                for ib in range(CB):
                    t = xpool.tile([128, HP, WP], F16, tag=f"x{ib}")
                    nc.sync.dma_start(out=t[:, :hh, :], in_=xp[img, ib, :, :hh, :])
                    nc.sync.dma_start(out=t[:, hh:, :], in_=xp[img, ib, :, hh:, :])
                    xt.append(t)
                return xt

            def wino_groups(vt, ws, ob, r0, nr):
                ps = []
                for b in range(4):
                    p = pspool.tile([128, nr, T], F32, name="ps", tag="ps")
                    k = 0
                    for ib in range(CB):
                        for ky in range(3):
                            nc.tensor.matmul(
                                p[:],
                                ws[ib][:, 3 * b + ky, 128 * ob:128 * ob + 128],
                                vt[ib][:, b, r0 + ky:r0 + ky + nr, :],
                                start=(k == 0), stop=(k == 5))
                            k += 1
                    ps.append(p)
                return ps

            def combine(ps, nr):
                # y_even = m0+m1+m2, y_odd = m1-m2-m3. Scalar copies the
                # shared m1/m2 to SBUF fp16; ops touching PSUM (fp32) stay
                # on the DVE, fp16-only ones go to the Pool engine.
                t1 = tpool.tile([128, nr, T], F16, name="t1")
                t2 = tpool.tile([128, nr, T], F16, name="t2")
                nc.scalar.activation(t1[:], ps[1][:], Copy)
                nc.scalar.activation(t2[:], ps[2][:], Copy)
                e1 = tpool.tile([128, nr, T], F16, name="e1")
                nc.vector.tensor_tensor(out=e1[:], in0=ps[0][:], in1=t1[:],
                                        op=Add)
                o1 = tpool.tile([128, nr, T], F16, name="o1")
                nc.gpsimd.tensor_tensor(out=o1[:], in0=t1[:], in1=t2[:], op=Sub)
                e = tpool.tile([128, nr, T], F16, name="e")
                nc.gpsimd.tensor_tensor(out=e[:], in0=e1[:], in1=t2[:], op=Add)
                o = tpool.tile([128, nr, T], F16, name="o")
                nc.vector.tensor_tensor(out=o[:], in0=o1[:], in1=ps[3][:],
                                        op=Sub)
                return e, o

            def conv1(img):
                ht = [hts_all[(img % 2, ob)] for ob in range(CB)]
                for ob in range(CB):
                    for c in range(NCHUNK):
                        r0 = RC * c
                        ps = wino_groups(v1s, w1s, ob, r0, RC)
                        e, o = combine(ps, RC)
                        # even outputs j=2t land on odd padded cols (the "O"
                        # region); odd outputs on even padded cols E[t+1]
                        nc.scalar.activation(
                            ht[ob][:, 1 + r0:1 + r0 + RC, OO0], e[:],
                            Relu, bias=b1s[ob][:], scale=1.0)
                        nc.scalar.activation(
                            ht[ob][:, 1 + r0:1 + r0 + RC, EO1], o[:],
                            Relu, bias=b1s[ob][:], scale=1.0)
                return ht

            def conv2(img, xt):
                for ob in range(CB):
                    for c in range(NCHUNK):
                        r0 = RC * c
                        # the very last group sits on the critical path (its
                        # epilogue chain + DMA is fully serial); split it so
                        # the first half's epilogue overlaps the second half
                        last = (img == NPC - 1 and ob == CB - 1
                                and c == NCHUNK - 1)
                        parts = [(r0, RC // 2), (r0 + RC // 2, RC - RC // 2)] \
                            if last else [(r0, RC)]
                        for pr0, nr in parts:
                            ps = wino_groups(v2s, w2s, ob, pr0, nr)
                            e, o = combine(ps, nr)
                            rr = slice(1 + pr0, 1 + pr0 + nr)
                            e2 = tpool.tile([128, nr, T], F16, name="e2")
                            nc.gpsimd.tensor_tensor(out=e2[:], in0=e[:],
                                                    in1=xt[ob][:, rr, OO0],
                                                    op=Add)
                            o2 = tpool.tile([128, nr, T], F16, name="o2")
                            nc.gpsimd.tensor_tensor(out=o2[:], in0=o[:],
                                                    in1=xt[ob][:, rr, EO1],
                                                    op=Add)
                            yt = ypool.tile([128, nr, W], F32, tag=f"y{ob}",
                                            name="yt")
                            nc.scalar.activation(yt[:, :, 0:56:2], e2[:],
                                                 Relu, bias=b2s[ob][:],
                                                 scale=1.0)
                            nc.scalar.activation(yt[:, :, 1:56:2], o2[:],
                                                 Relu, bias=b2s[ob][:],
                                                 scale=1.0)
                            nc.sync.dma_start(
                                out=y[img, ob, :, pr0:pr0 + nr, :], in_=yt[:])

            # software pipeline: conv1(i+1) emitted before conv2(i) so the PE
            # has independent work while conv2(i) waits on its V2 transform
            xts = {0: xt0}
            vtransform(v1s, xt0, 0, hh)   # split so it starts right after
            vtransform(v1s, xt0, hh, HP)  # the first half-tile DMAs land
            conv1(0)
            load_w2()
            for img in range(1, NPC):
                xts[img] = load_x(img)
                vtransform(v1s, xts[img])
                conv1(img)
                vtransform(v2s, [hts_all[((img - 1) % 2, ob)]
                                 for ob in range(CB)])
                conv2(img - 1, xts[img - 1])
            vtransform(v2s, [hts_all[((NPC - 1) % 2, ob)] for ob in range(CB)])
            conv2(NPC - 1, xts[NPC - 1])

    nc.compile()
    return nc


def _prep(inputs):
    x = np.asarray(inputs["x"], np.float32)
    G = np.array([[1, 0, 0], [.5, .5, .5], [.5, -.5, .5], [0, 0, 1]],
                 np.float32)
    out = {}
    for i in (1, 2):
        s = np.asarray(inputs[f"g{i}"], np.float32) / np.sqrt(
            np.asarray(inputs[f"rv{i}"], np.float32) + EPS)
        b = (np.asarray(inputs[f"b{i}"], np.float32)
             - np.asarray(inputs[f"rm{i}"], np.float32) * s)
        w = np.asarray(inputs[f"w{i}"], np.float32) * s[:, None, None, None]
        # winograd weight transform along kx: wwin[o,i,ky,b] = G[b,:] . w[o,i,ky,:]
        wwin = np.einsum('bk,oiyk->oiyb', G, w)
        # layout [CB, 128, 12, O] with tap index 3*b+ky
        wt = np.ascontiguousarray(wwin.transpose(1, 3, 2, 0)).reshape(
            C, 12, C).reshape(CB, 128, 12, C).astype(np.float16)
        out[f"w{i}t"] = wt
        out[f"b{i}"] = np.ascontiguousarray(b.reshape(CB, 128, 1))
    xpad = np.zeros((N, C, HP, WP), np.float32)
    xpad[:, :, 1:-1, 1:-1] = x
    # de-interleave columns: [29 even cols 0,2,..,56 | 29 odd cols 1,..,57]
    xd = np.concatenate([xpad[..., 0::2], xpad[..., 1::2]],
                        axis=-1).astype(np.float16)
    out["xp"] = np.ascontiguousarray(xd).reshape(NCORES, NPC, CB, 128, HP, WP)
    return out


def run(inputs, trace=False):
    if "nc" not in _CACHE:
        _CACHE["nc"] = _build()
    nc = _CACHE["nc"]
    p = _prep(inputs)
    in_maps = [{"xp": p["xp"][c], "w1t": p["w1t"], "w2t": p["w2t"],
                "b1": p["b1"], "b2": p["b2"]} for c in range(NCORES)]
    res = run_bass_kernel_spmd(nc, in_maps, core_ids=list(range(NCORES)),
                               trace=trace)
    yout = np.concatenate(
        [r["y"].reshape(NPC, C, H, W) for r in res.results], axis=0)
    return yout, res


def kernel(**inputs):
    yout, _ = run(inputs)
    return yout


# revision 5
# speedup vs baseline: 1.2039x; 1.2039x over previous
"""ResNet BasicBlock (conv3x3-BN-ReLU-conv3x3-BN-add-ReLU) on 8 Trainium2 cores.

Strategy:
  - Pure data parallel: batch 32 -> 4 images per core; weights/BN replicated.
  - BN folded into conv weights on host; bias applied in the epilogue.
  - 1D Winograd F(2,3) along W: 1.5x fewer PE MACs than direct conv.
      V_b = (B^T d)_b  computed on DVE as slice add/subs of the padded input
      m_b = sum_{ky,ib} wtil_b[ky]^T @ V_b   (PE, PSUM fp32; 24 matmuls x
            392 cols per 14-row chunk vs direct 18 x 784)
      y_even = m0+m1+m2, y_odd = m1-m2-m3   (Scalar copies the shared m1/m2
            to SBUF fp16; DVE/Pool combine; m0/m3 read straight from PSUM)
    Weight transform (G along kx) and BN fold are done on host.
  - Columns stored DE-INTERLEAVED (all 29 even cols, then all 29 odd cols)
    for x and h, so every V-transform/residual slice is contiguous fp16 ->
    DVE runs them in 2x_1p mode (the only fast mode TensorTensor supports).
  - Epilogue combine ops that touch PSUM (fp32) stay on the DVE; the
    fp16-only ones (o1, e, residual adds) go to the otherwise-idle Pool
    (gpsimd) engine so the DVE keeps up with the PE.
  - fp16 matmul operands, fp32 PSUM accumulation (rel err ~1e-3, gate 2e-2).
"""

import numpy as np

import concourse.mybir as mybir
import concourse.tile as tile
from concourse import bacc
from concourse.bass_utils import run_bass_kernel_spmd

EPS = 1e-5
NCORES = 8
N, C, H, W = 32, 256, 56, 56
NPC = N // NCORES          # images per core
HP, WP = H + 2, W + 2      # padded spatial
CB = C // 128              # channel blocks (2)
RC = 14                    # rows per PSUM chunk
NCHUNK = H // RC           # 4 chunks
T = W // 2                 # winograd tiles per row (28)
F16 = mybir.dt.float16
F32 = mybir.dt.float32

_CACHE = {}


def _build():
    nc = bacc.Bacc("TRN2", target_bir_lowering=False, debug=False,
                   num_devices=NCORES)
    xp = nc.dram_tensor("xp", [NPC, CB, 128, HP, WP], F16,
                        kind="ExternalInput").ap()
    w1t = nc.dram_tensor("w1t", [CB, 128, 12, C], F16, kind="ExternalInput").ap()
    w2t = nc.dram_tensor("w2t", [CB, 128, 12, C], F16, kind="ExternalInput").ap()
    b1 = nc.dram_tensor("b1", [CB, 128, 1], F32, kind="ExternalInput").ap()
    b2 = nc.dram_tensor("b2", [CB, 128, 1], F32, kind="ExternalInput").ap()
    y = nc.dram_tensor("y", [NPC, CB, 128, H, W], F32,
                       kind="ExternalOutput").ap()

    Relu = mybir.ActivationFunctionType.Relu
    Copy = mybir.ActivationFunctionType.Copy
    Add = mybir.AluOpType.add
    Sub = mybir.AluOpType.subtract

    # de-interleaved column regions of a padded 58-col row:
    # cols [0:29) = original even cols 0,2,..,56 ("E"), [29:58) = odd 1,..,57
    EO0, EO1 = slice(0, 28), slice(1, 29)        # E[t], E[t+1]
    OO0, OO1 = slice(29, 57), slice(30, 58)      # O[t], O[t+1]

    with tile.TileContext(nc) as tc:
        with tc.tile_pool(name="w", bufs=1) as wp, \
             tc.tile_pool(name="x", bufs=3) as xpool, \
             tc.tile_pool(name="v", bufs=1) as vpool, \
             tc.tile_pool(name="h", bufs=1) as hpool, \
             tc.tile_pool(name="yst", bufs=3) as ypool, \
             tc.tile_pool(name="tmp", bufs=3) as tpool, \
             tc.tile_pool(name="ps", bufs=8, space="PSUM") as pspool:

            # Startup: DMA issues serialize at ~620ns each on the Sync queue,
            # so order by first-need: x halves ib-interleaved (they feed the
            # V transform, the longest dependency chain), then weights.
            hh = HP // 2
            w1s, w2s, b1s, b2s = [], [], [], []
            xt0 = [xpool.tile([128, HP, WP], F16, tag=f"x{ib}", name=f"xt0_{ib}")
                   for ib in range(CB)]
            for ib in range(CB):
                nc.sync.dma_start(out=xt0[ib][:, :hh, :],
                                  in_=xp[0, ib, :, :hh, :])
            for ib in range(CB):
                nc.sync.dma_start(out=xt0[ib][:, hh:, :],
                                  in_=xp[0, ib, :, hh:, :])
            for ib in range(CB):
                t = wp.tile([128, 12, C], F16, tag=f"w1_{ib}")
                w1s.append(t)
                nc.sync.dma_start(out=w1s[ib][:, :, :128],
                                  in_=w1t[ib, :, :, :128])
            for ib in range(CB):
                nc.sync.dma_start(out=w1s[ib][:, :, 128:],
                                  in_=w1t[ib, :, :, 128:])
                t = wp.tile([128, 1], F32, tag=f"b1_{ib}")
                nc.sync.dma_start(out=t[:], in_=b1[ib])
                b1s.append(t)

            def load_w2():
                for ib in range(CB):
                    t = wp.tile([128, 12, C], F16, tag=f"w2_{ib}")
                    nc.sync.dma_start(out=t[:], in_=w2t[ib])
                    w2s.append(t)
                    t = wp.tile([128, 1], F32, tag=f"b2_{ib}")
                    nc.sync.dma_start(out=t[:], in_=b2[ib])
                    b2s.append(t)

            # PE warmup: the HAM clock gate holds the PE at 1.2 GHz until it
            # has been busy ~3.4us. The PE is idle during the initial DMA +
            # V-transform wait anyway, so run throwaway matmuls on a zeroed
            # scratch tile to unthrottle the clock before the first real one.
            scratch = wp.tile([128, RC * T], F16, tag="warm_scratch")
            nc.gpsimd.memset(scratch[:], 0.0)
            ps_w = pspool.tile([128, RC * T], F32, name="ps_warm", tag="ps")
            for _ in range(16):
                nc.tensor.matmul(ps_w[:], scratch[:, :128], scratch[:],
                                 start=True, stop=True)

            # persistent conv1-output tiles (de-interleaved cols, padded),
            # 2 channel blocks x 2 pipeline parities, fully zeroed ONCE (the
            # border cols/rows must stay zero); interior rewritten per image.
            hts_all = {}
            for par in range(2):
                for ob in range(CB):
                    t = hpool.tile([128, HP, WP], F16, tag=f"h{par}_{ob}")
                    nc.gpsimd.memset(t[:], 0.0)
                    hts_all[(par, ob)] = t

            # persistent Winograd-transform tiles (single-buffered: the
            # transform for the next use WAR-waits on the previous conv's
            # reads; the DVE has slack to hide that)
            v1s = [vpool.tile([128, 4, HP, T], F16, tag=f"v1_{ib}",
                              name=f"v1_{ib}") for ib in range(CB)]
            v2s = [vpool.tile([128, 4, HP, T], F16, tag=f"v2_{ib}",
                              name=f"v2_{ib}") for ib in range(CB)]

            def vtransform(vt, src, r0=0, r1=HP):
                # V_b = (B^T d)_b over W for padded rows [r0, r1), as
                # contiguous slice ops on the de-interleaved layout
                # (fp16 packed -> DVE 2x_1p mode)
                r = slice(r0, r1)
                for ib in range(CB):
                    s, v = src[ib], vt[ib]
                    nc.vector.tensor_tensor(out=v[:, 0, r], in0=s[:, r, EO0],
                                            in1=s[:, r, EO1], op=Sub)
                    nc.vector.tensor_tensor(out=v[:, 1, r], in0=s[:, r, OO0],
                                            in1=s[:, r, EO1], op=Add)
                    nc.vector.tensor_tensor(out=v[:, 2, r], in0=s[:, r, EO1],
                                            in1=s[:, r, OO0], op=Sub)
                    nc.vector.tensor_tensor(out=v[:, 3, r], in0=s[:, r, OO0],
                                            in1=s[:, r, OO1], op=Sub)

            def load_x(img):
                xt = []
                for ib in range(CB):
                    t = xpool.tile([128, HP, WP], F16, tag=f"x{ib}")
                    nc.sync.dma_start(out=t[:, :hh, :], in_=xp[img, ib, :, :hh, :])
                    nc.sync.dma_start(out=t[:, hh:, :], in_=xp[img, ib, :, hh:, :])
                    xt.append(t)
                return xt

            def wino_groups(vt, ws, ob, r0, nr):
                ps = []
                for b in range(4):
                    p = pspool.tile([128, nr, T], F32, name="ps", tag="ps")
                    k = 0
                    for ib in range(CB):
                        for ky in range(3):
                            nc.tensor.matmul(
                                p[:],
                                ws[ib][:, 3 * b + ky, 128 * ob:128 * ob + 128],
                                vt[ib][:, b, r0 + ky:r0 + ky + nr, :],
                                start=(k == 0), stop=(k == 5))
                            k += 1
                    ps.append(p)
                return ps

            def combine(ps, nr):
                # y_even = m0+m1+m2, y_odd = m1-m2-m3. Scalar copies the
                # shared m1/m2 to SBUF fp16; ops touching PSUM (fp32) stay
                # on the DVE, fp16-only ones go to the Pool engine.
                t1 = tpool.tile([128, nr, T], F16, name="t1")
                t2 = tpool.tile([128, nr, T], F16, name="t2")
                nc.scalar.activation(t1[:], ps[1][:], Copy)
                nc.scalar.activation(t2[:], ps[2][:], Copy)
                e1 = tpool.tile([128, nr, T], F16, name="e1")
                nc.vector.tensor_tensor(out=e1[:], in0=ps[0][:], in1=t1[:],
                                        op=Add)
                o1 = tpool.tile([128, nr, T], F16, name="o1")
                nc.vector.tensor_tensor(out=o1[:], in0=t1[:], in1=t2[:], op=Sub)
                e = tpool.tile([128, nr, T], F16, name="e")
                nc.vector.tensor_tensor(out=e[:], in0=e1[:], in1=t2[:], op=Add)
                o = tpool.tile([128, nr, T], F16, name="o")
                nc.vector.tensor_tensor(out=o[:], in0=o1[:], in1=ps[3][:],
                                        op=Sub)
                return e, o

            def conv1(img):
                ht = [hts_all[(img % 2, ob)] for ob in range(CB)]
                for ob in range(CB):
                    for c in range(NCHUNK):
                        r0 = RC * c
                        ps = wino_groups(v1s, w1s, ob, r0, RC)
                        e, o = combine(ps, RC)
                        # even outputs j=2t land on odd padded cols (the "O"
                        # region); odd outputs on even padded cols E[t+1]
                        nc.scalar.activation(
                            ht[ob][:, 1 + r0:1 + r0 + RC, OO0], e[:],
                            Relu, bias=b1s[ob][:], scale=1.0)
                        nc.scalar.activation(
                            ht[ob][:, 1 + r0:1 + r0 + RC, EO1], o[:],
                            Relu, bias=b1s[ob][:], scale=1.0)
                return ht

            def conv2(img, xt):
                for ob in range(CB):
                    for c in range(NCHUNK):
                        r0 = RC * c
                        # the very last group sits on the critical path (its
                        # epilogue chain + DMA is fully serial); split it so
                        # the first half's epilogue overlaps the second half
                        last = (img == NPC - 1 and ob == CB - 1
                                and c == NCHUNK - 1)
                        parts = [(r0, RC // 2), (r0 + RC // 2, RC - RC // 2)] \
                            if last else [(r0, RC)]
                        for pr0, nr in parts:
                            ps = wino_groups(v2s, w2s, ob, pr0, nr)
                            e, o = combine(ps, nr)
                            rr = slice(1 + pr0, 1 + pr0 + nr)
                            e2 = tpool.tile([128, nr, T], F16, name="e2")
                            nc.gpsimd.tensor_tensor(out=e2[:], in0=e[:],
                                                    in1=xt[ob][:, rr, OO0],
                                                    op=Add)
                            o2 = tpool.tile([128, nr, T], F16, name="o2")
                            nc.gpsimd.tensor_tensor(out=o2[:], in0=o[:],
                                                    in1=xt[ob][:, rr, EO1],
                                                    op=Add)
                            yt = ypool.tile([128, nr, W], F32, tag=f"y{ob}",
                                            name="yt")
                            nc.scalar.activation(yt[:, :, 0:56:2], e2[:],
                                                 Relu, bias=b2s[ob][:],
                                                 scale=1.0)
                            nc.scalar.activation(yt[:, :, 1:56:2], o2[:],
                                                 Relu, bias=b2s[ob][:],
                                                 scale=1.0)
                            nc.sync.dma_start(
                                out=y[img, ob, :, pr0:pr0 + nr, :], in_=yt[:])

            # software pipeline: conv1(i+1) emitted before conv2(i) so the PE
            # has independent work while conv2(i) waits on its V2 transform
            xts = {0: xt0}
            vtransform(v1s, xt0, 0, hh)   # split so it starts right after
            vtransform(v1s, xt0, hh, HP)  # the first half-tile DMAs land
            conv1(0)
            load_w2()
            for img in range(1, NPC):
                xts[img] = load_x(img)
                vtransform(v1s, xts[img])
                conv1(img)
                vtransform(v2s, [hts_all[((img - 1) % 2, ob)]
                                 for ob in range(CB)])
                conv2(img - 1, xts[img - 1])
            vtransform(v2s, [hts_all[((NPC - 1) % 2, ob)] for ob in range(CB)])
            conv2(NPC - 1, xts[NPC - 1])

    nc.compile()
    return nc


def _prep(inputs):
    x = np.asarray(inputs["x"], np.float32)
    G = np.array([[1, 0, 0], [.5, .5, .5], [.5, -.5, .5], [0, 0, 1]],
                 np.float32)
    out = {}
    for i in (1, 2):
        s = np.asarray(inputs[f"g{i}"], np.float32) / np.sqrt(
            np.asarray(inputs[f"rv{i}"], np.float32) + EPS)
        b = (np.asarray(inputs[f"b{i}"], np.float32)
             - np.asarray(inputs[f"rm{i}"], np.float32) * s)
        w = np.asarray(inputs[f"w{i}"], np.float32) * s[:, None, None, None]
        # winograd weight transform along kx: wwin[o,i,ky,b] = G[b,:] . w[o,i,ky,:]
        wwin = np.einsum('bk,oiyk->oiyb', G, w)
        # layout [CB, 128, 12, O] with tap index 3*b+ky
        wt = np.ascontiguousarray(wwin.transpose(1, 3, 2, 0)).reshape(
            C, 12, C).reshape(CB, 128, 12, C).astype(np.float16)
        out[f"w{i}t"] = wt
        out[f"b{i}"] = np.ascontiguousarray(b.reshape(CB, 128, 1))
    xpad = np.zeros((N, C, HP, WP), np.float32)
    xpad[:, :, 1:-1, 1:-1] = x
    # de-interleave columns: [29 even cols 0,2,..,56 | 29 odd cols 1,..,57]
    xd = np.concatenate([xpad[..., 0::2], xpad[..., 1::2]],
                        axis=-1).astype(np.float16)
    out["xp"] = np.ascontiguousarray(xd).reshape(NCORES, NPC, CB, 128, HP, WP)
    return out


def run(inputs, trace=False):
    if "nc" not in _CACHE:
        _CACHE["nc"] = _build()
    nc = _CACHE["nc"]
    p = _prep(inputs)
    in_maps = [{"xp": p["xp"][c], "w1t": p["w1t"], "w2t": p["w2t"],
                "b1": p["b1"], "b2": p["b2"]} for c in range(NCORES)]
    res = run_bass_kernel_spmd(nc, in_maps, core_ids=list(range(NCORES)),
                               trace=trace)
    yout = np.concatenate(
        [r["y"].reshape(NPC, C, H, W) for r in res.results], axis=0)
    return yout, res


def kernel(**inputs):
    yout, _ = run(inputs)
    return yout


# revision 7
# speedup vs baseline: 1.2962x; 1.0767x over previous
"""ResNet BasicBlock (conv3x3-BN-ReLU-conv3x3-BN-add-ReLU) on 8 Trainium2 cores.

Strategy:
  - Pure data parallel: batch 32 -> 4 images per core; weights/BN replicated.
  - BN folded into conv weights on host; bias applied in the epilogue.
  - 1D Winograd F(2,3) along W: 1.5x fewer PE MACs than direct conv.
      V_b = (B^T d)_b  computed on DVE as slice add/subs of the padded input
      m_b = sum_{ky,ib} wtil_b[ky]^T @ V_b   (PE, PSUM fp32; 24 matmuls x
            392 cols per 14-row chunk vs direct 18 x 784)
      y_even = m0+m1+m2, y_odd = m1-m2-m3   (Scalar copies the shared m1/m2
            to SBUF fp16; DVE combines, reading m0/m3 straight from PSUM)
    Weight transform (G along kx) and BN fold are done on host.
  - Columns stored DE-INTERLEAVED (all 29 even cols, then all 29 odd cols)
    for x and h, so every V-transform/residual slice is contiguous fp16 ->
    DVE runs them in 2x_1p mode (the only fast mode TensorTensor supports).
  - Startup is DMA-latency-bound: x and w1 are loaded in fine-grained
    first-need pieces (w layout is ob-major so the first output-half's
    weights land first), the first image's V transform runs banded behind
    the row bands, and 16 PE warmup matmuls hide the wait + HAM clock ramp.
  - V2 tiles are double-buffered by image parity so the transform for
    image i (emitted after conv1(i)) WAR-depends only on conv2(i-2) and
    hides completely under conv2(i-1)'s matmuls.
  - fp16 matmul operands, fp32 PSUM accumulation (rel err ~1e-3, gate 2e-2).
"""

import numpy as np

import concourse.mybir as mybir
import concourse.tile as tile
from concourse import bacc
from concourse.bass_utils import run_bass_kernel_spmd

EPS = 1e-5
NCORES = 8
N, C, H, W = 32, 256, 56, 56
NPC = N // NCORES          # images per core
HP, WP = H + 2, W + 2      # padded spatial
CB = C // 128              # channel blocks (2)
RC = 14                    # rows per PSUM chunk
NCHUNK = H // RC           # 4 chunks
T = W // 2                 # winograd tiles per row (28)
F16 = mybir.dt.float16
F32 = mybir.dt.float32
XBANDS = [(0, 16), (16, 37), (37, 58)]   # startup x row bands

_CACHE = {}


def _build():
    nc = bacc.Bacc("TRN2", target_bir_lowering=False, debug=False,
                   num_devices=NCORES)
    xp = nc.dram_tensor("xp", [NPC, CB, 128, HP, WP], F16,
                        kind="ExternalInput").ap()
    w1t = nc.dram_tensor("w1t", [CB, 128, CB, 12, 128], F16,
                         kind="ExternalInput").ap()
    w2t = nc.dram_tensor("w2t", [CB, 128, CB, 12, 128], F16,
                         kind="ExternalInput").ap()
    b1 = nc.dram_tensor("b1", [CB, 128, 1], F32, kind="ExternalInput").ap()
    b2 = nc.dram_tensor("b2", [CB, 128, 1], F32, kind="ExternalInput").ap()
    y = nc.dram_tensor("y", [NPC, CB, 128, H, W], F32,
                       kind="ExternalOutput").ap()

    Relu = mybir.ActivationFunctionType.Relu
    Copy = mybir.ActivationFunctionType.Copy
    Add = mybir.AluOpType.add
    Sub = mybir.AluOpType.subtract

    # de-interleaved column regions of a padded 58-col row:
    # cols [0:29) = original even cols 0,2,..,56 ("E"), [29:58) = odd 1,..,57
    EO0, EO1 = slice(0, 28), slice(1, 29)        # E[t], E[t+1]
    OO0, OO1 = slice(29, 57), slice(30, 58)      # O[t], O[t+1]

    with tile.TileContext(nc) as tc:
        with tc.tile_pool(name="w", bufs=1) as wp, \
             tc.tile_pool(name="x", bufs=3) as xpool, \
             tc.tile_pool(name="v", bufs=1) as vpool, \
             tc.tile_pool(name="h", bufs=1) as hpool, \
             tc.tile_pool(name="yst", bufs=2) as ypool, \
             tc.tile_pool(name="tmp", bufs=2) as tpool, \
             tc.tile_pool(name="ps", bufs=8, space="PSUM") as pspool:

            # Startup: DMA issues serialize at ~620ns each on the Sync queue
            # and the first matmul is gated by DMA completion, so load x row
            # bands and per-(ib,ob) weight pieces in strict first-need order.
            w1s, w2s, b1s, b2s = [], [], [], []
            xt0 = [xpool.tile([128, HP, WP], F16, tag=f"x{ib}", name=f"xt0_{ib}")
                   for ib in range(CB)]
            for ib in range(CB):
                t = wp.tile([128, CB, 12, 128], F16, tag=f"w1_{ib}")
                w1s.append(t)
            r0b, r1b = XBANDS[0]
            for ib in range(CB):
                nc.sync.dma_start(out=xt0[ib][:, r0b:r1b, :],
                                  in_=xp[0, ib, :, r0b:r1b, :])
            for ib in range(CB):
                nc.sync.dma_start(out=w1s[ib][:, 0], in_=w1t[ib, :, 0])
            r0b, r1b = XBANDS[1]
            for ib in range(CB):
                nc.sync.dma_start(out=xt0[ib][:, r0b:r1b, :],
                                  in_=xp[0, ib, :, r0b:r1b, :])
            for ib in range(CB):
                t = wp.tile([128, 1], F32, tag=f"b1_{ib}")
                nc.sync.dma_start(out=t[:], in_=b1[ib])
                b1s.append(t)
            r0b, r1b = XBANDS[2]
            for ib in range(CB):
                nc.sync.dma_start(out=xt0[ib][:, r0b:r1b, :],
                                  in_=xp[0, ib, :, r0b:r1b, :])
            for ib in range(CB):
                nc.sync.dma_start(out=w1s[ib][:, 1], in_=w1t[ib, :, 1])

            def load_w2():
                for ib in range(CB):
                    t = wp.tile([128, CB, 12, 128], F16, tag=f"w2_{ib}")
                    nc.sync.dma_start(out=t[:], in_=w2t[ib])
                    w2s.append(t)
                    t = wp.tile([128, 1], F32, tag=f"b2_{ib}")
                    nc.sync.dma_start(out=t[:], in_=b2[ib])
                    b2s.append(t)

            # PE warmup: the HAM clock gate holds the PE at 1.2 GHz until it
            # has been busy ~3.4us. The PE is idle during the initial DMA +
            # V-transform wait anyway, so run throwaway matmuls on a zeroed
            # scratch tile to unthrottle the clock before the first real one.
            scratch = wp.tile([128, RC * T], F16, tag="warm_scratch")
            nc.gpsimd.memset(scratch[:], 0.0)
            ps_w = pspool.tile([128, RC * T], F32, name="ps_warm", tag="ps")
            for _ in range(16):
                nc.tensor.matmul(ps_w[:], scratch[:, :128], scratch[:],
                                 start=True, stop=True)

            # persistent conv1-output tiles (de-interleaved cols, padded),
            # 2 channel blocks x 2 pipeline parities, fully zeroed ONCE (the
            # border cols/rows must stay zero); interior rewritten per image.
            hts_all = {}
            for par in range(2):
                for ob in range(CB):
                    t = hpool.tile([128, HP, WP], F16, tag=f"h{par}_{ob}")
                    nc.gpsimd.memset(t[:], 0.0)
                    hts_all[(par, ob)] = t

            # Winograd-transform tiles. V1 single-buffered (its transform
            # hides under the previous image's conv2); V2 double-buffered by
            # parity so v2tf(i) WAR-depends only on conv2(i-2).
            v1s = [vpool.tile([128, 4, HP, T], F16, tag=f"v1_{ib}",
                              name=f"v1_{ib}") for ib in range(CB)]
            v2s_all = {(par, ib): vpool.tile([128, 4, HP, T], F16,
                                             tag=f"v2_{par}_{ib}",
                                             name=f"v2_{par}_{ib}")
                       for par in range(2) for ib in range(CB)}

            def vtransform(vt, src, r0=0, r1=HP, order=None):
                # V_b = (B^T d)_b over W for padded rows [r0, r1), as
                # contiguous slice ops on the de-interleaved layout
                # (fp16 packed -> DVE 2x_1p mode)
                r = slice(r0, r1)
                for ib in (order or range(CB)):
                    s, v = src[ib], vt[ib]
                    nc.vector.tensor_tensor(out=v[:, 0, r], in0=s[:, r, EO0],
                                            in1=s[:, r, EO1], op=Sub)
                    nc.vector.tensor_tensor(out=v[:, 1, r], in0=s[:, r, OO0],
                                            in1=s[:, r, EO1], op=Add)
                    nc.vector.tensor_tensor(out=v[:, 2, r], in0=s[:, r, EO1],
                                            in1=s[:, r, OO0], op=Sub)
                    nc.vector.tensor_tensor(out=v[:, 3, r], in0=s[:, r, OO0],
                                            in1=s[:, r, OO1], op=Sub)

            def load_x(img):
                xt = []
                for ib in range(CB):
                    t = xpool.tile([128, HP, WP], F16, tag=f"x{ib}")
                    hh = HP // 2
                    nc.sync.dma_start(out=t[:, :hh, :], in_=xp[img, ib, :, :hh, :])
                    nc.sync.dma_start(out=t[:, hh:, :], in_=xp[img, ib, :, hh:, :])
                    xt.append(t)
                return xt

            def wino_groups(vt, ws, ob, r0, nr):
                ps = []
                for b in range(4):
                    p = pspool.tile([128, nr, T], F32, name="ps", tag="ps")
                    k = 0
                    for ib in range(CB):
                        for ky in range(3):
                            nc.tensor.matmul(
                                p[:],
                                ws[ib][:, ob, 3 * b + ky, :],
                                vt[ib][:, b, r0 + ky:r0 + ky + nr, :],
                                start=(k == 0), stop=(k == 5))
                            k += 1
                    ps.append(p)
                return ps

            def combine(ps, nr):
                # y_even = m0+m1+m2, y_odd = m1-m2-m3. Scalar copies the
                # shared m1/m2 to SBUF fp16; DVE combines (m0/m3 read
                # straight from PSUM; fp16-only ops run in 2x_1p mode).
                t1 = tpool.tile([128, nr, T], F16, name="t1")
                t2 = tpool.tile([128, nr, T], F16, name="t2")
                nc.scalar.activation(t1[:], ps[1][:], Copy)
                nc.scalar.activation(t2[:], ps[2][:], Copy)
                e1 = tpool.tile([128, nr, T], F16, name="e1")
                nc.vector.tensor_tensor(out=e1[:], in0=ps[0][:], in1=t1[:],
                                        op=Add)
                o1 = tpool.tile([128, nr, T], F16, name="o1")
                nc.vector.tensor_tensor(out=o1[:], in0=t1[:], in1=t2[:], op=Sub)
                e = tpool.tile([128, nr, T], F16, name="e")
                nc.vector.tensor_tensor(out=e[:], in0=e1[:], in1=t2[:], op=Add)
                o = tpool.tile([128, nr, T], F16, name="o")
                nc.vector.tensor_tensor(out=o[:], in0=o1[:], in1=ps[3][:],
                                        op=Sub)
                return e, o

            def conv1(img, vt):
                ht = [hts_all[(img % 2, ob)] for ob in range(CB)]
                for ob in range(CB):
                    for c in range(NCHUNK):
                        r0 = RC * c
                        ps = wino_groups(vt, w1s, ob, r0, RC)
                        e, o = combine(ps, RC)
                        # even outputs j=2t land on odd padded cols (the "O"
                        # region); odd outputs on even padded cols E[t+1]
                        nc.scalar.activation(
                            ht[ob][:, 1 + r0:1 + r0 + RC, OO0], e[:],
                            Relu, bias=b1s[ob][:], scale=1.0)
                        nc.scalar.activation(
                            ht[ob][:, 1 + r0:1 + r0 + RC, EO1], o[:],
                            Relu, bias=b1s[ob][:], scale=1.0)
                return ht

            def conv2(img, xt, vt):
                for ob in range(CB):
                    for c in range(NCHUNK):
                        r0 = RC * c
                        # the very last group sits on the critical path (its
                        # epilogue chain + DMA is fully serial); split it so
                        # the first half's epilogue overlaps the second half
                        last = (img == NPC - 1 and ob == CB - 1
                                and c == NCHUNK - 1)
                        parts = [(r0, RC // 2), (r0 + RC // 2, RC - RC // 2)] \
                            if last else [(r0, RC)]
                        for pr0, nr in parts:
                            ps = wino_groups(vt, w2s, ob, pr0, nr)
                            e, o = combine(ps, nr)
                            rr = slice(1 + pr0, 1 + pr0 + nr)
                            e2 = tpool.tile([128, nr, T], F16, name="e2")
                            nc.vector.tensor_tensor(out=e2[:], in0=e[:],
                                                    in1=xt[ob][:, rr, OO0],
                                                    op=Add)
                            o2 = tpool.tile([128, nr, T], F16, name="o2")
                            nc.vector.tensor_tensor(out=o2[:], in0=o[:],
                                                    in1=xt[ob][:, rr, EO1],
                                                    op=Add)
                            yt = ypool.tile([128, nr, W], F32, tag="y",
                                            name="yt")
                            nc.scalar.activation(yt[:, :, 0:56:2], e2[:],
                                                 Relu, bias=b2s[ob][:],
                                                 scale=1.0)
                            nc.scalar.activation(yt[:, :, 1:56:2], o2[:],
                                                 Relu, bias=b2s[ob][:],
                                                 scale=1.0)
                            nc.sync.dma_start(
                                out=y[img, ob, :, pr0:pr0 + nr, :], in_=yt[:])

            # first image's V transform runs banded right behind the x DMAs
            # (ib-alternating inside each band so work starts ASAP)
            for r0b, r1b in XBANDS:
                vtransform(v1s, xt0, r0b, r1b)

            # software pipeline: conv1(i+1) emitted before conv2(i) so the PE
            # has independent work while conv2(i) waits on its V2 transform
            xts = {0: xt0}
            xts[1] = load_x(1)
            conv1(0, v1s)
            load_w2()
            for img in range(1, NPC):
                if img + 1 < NPC:
                    xts[img + 1] = load_x(img + 1)
                vtransform(v1s, xts[img])
                conv1(img, v1s)
                par = (img - 1) % 2
                v2 = [v2s_all[(par, ib)] for ib in range(CB)]
                vtransform(v2, [hts_all[(par, ob)] for ob in range(CB)])
                conv2(img - 1, xts[img - 1], v2)
            par = (NPC - 1) % 2
            v2 = [v2s_all[(par, ib)] for ib in range(CB)]
            vtransform(v2, [hts_all[(par, ob)] for ob in range(CB)])
            conv2(NPC - 1, xts[NPC - 1], v2)

    nc.compile()
    return nc


def _prep(inputs):
    x = np.asarray(inputs["x"], np.float32)
    G = np.array([[1, 0, 0], [.5, .5, .5], [.5, -.5, .5], [0, 0, 1]],
                 np.float32)
    out = {}
    for i in (1, 2):
        s = np.asarray(inputs[f"g{i}"], np.float32) / np.sqrt(
            np.asarray(inputs[f"rv{i}"], np.float32) + EPS)
        b = (np.asarray(inputs[f"b{i}"], np.float32)
             - np.asarray(inputs[f"rm{i}"], np.float32) * s)
        w = np.asarray(inputs[f"w{i}"], np.float32) * s[:, None, None, None]
        # winograd weight transform along kx: wwin[o,i,ky,b] = G[b,:] . w[o,i,ky,:]
        wwin = np.einsum('bk,oiyk->oiyb', G, w)
        # layout [CB, 128, CB(ob), 12, 128] with tap index 3*b+ky, ob-major
        # so the first output-half's weights are a contiguous DMA piece
        wt = np.ascontiguousarray(wwin.transpose(1, 3, 2, 0)).reshape(
            C, 12, C).reshape(C, 12, CB, 128).transpose(0, 2, 1, 3).reshape(
            CB, 128, CB, 12, 128).astype(np.float16)
        out[f"w{i}t"] = np.ascontiguousarray(wt)
        out[f"b{i}"] = np.ascontiguousarray(b.reshape(CB, 128, 1))
    xpad = np.zeros((N, C, HP, WP), np.float32)
    xpad[:, :, 1:-1, 1:-1] = x
    # de-interleave columns: [29 even cols 0,2,..,56 | 29 odd cols 1,..,57]
    xd = np.concatenate([xpad[..., 0::2], xpad[..., 1::2]],
                        axis=-1).astype(np.float16)
    out["xp"] = np.ascontiguousarray(xd).reshape(NCORES, NPC, CB, 128, HP, WP)
    return out


def run(inputs, trace=False):
    if "nc" not in _CACHE:
        _CACHE["nc"] = _build()
    nc = _CACHE["nc"]
    p = _prep(inputs)
    in_maps = [{"xp": p["xp"][c], "w1t": p["w1t"], "w2t": p["w2t"],
                "b1": p["b1"], "b2": p["b2"]} for c in range(NCORES)]
    res = run_bass_kernel_spmd(nc, in_maps, core_ids=list(range(NCORES)),
                               trace=trace)
    yout = np.concatenate(
        [r["y"].reshape(NPC, C, H, W) for r in res.results], axis=0)
    return yout, res


def kernel(**inputs):
    yout, _ = run(inputs)
    return yout
